# revision 16
# baseline (speedup 1.0000x reference)
"""Trainium2 Bass kernel for gnn_message_passing (nn_BFR_28089086116615).

v2 restructure. Sharding: receiver axis i (G=4096 -> 8 cores x 512).
Per-core per-block work: sigma^T[j,i] = sigmoid(sdb[i] + ssrc[j]) on ACT
(bias = per-partition ssrc chunk, input = sdb row broadcast living in PSUM),
gated by wT (bf16 DVE 2x), contracted on PE with stationary [1|h] bf16 groups
so rowsum lands in psum row 0. Node/merge MLPs + BN stay fp32 (accuracy).
BN is computed only in the transposed layout; the natural-layout copy of the
normalized h is produced by a fused PE "transpose" matmul whose extra moving
column also computes ssrc2, so the AllGather payload already contains block-2's
per-sender logit term. Local sigma-2 chunks run while the AllGather is in
flight. h0 runs in fp32 under the w1 DMA shadow.
"""
import sys
sys.path.insert(0, "/opt/trn_rl_repo")
import numpy as np
import ml_dtypes

import concourse.bass as bass
import concourse.bacc as bacc
import concourse.mybir as mybir
import concourse.tile as tile
from concourse.bass_utils import run_bass_kernel_spmd

NC = 8
B, G, NI, H, NO = 2, 4096, 8, 32, 32
GL = G // NC              # 512 local receivers per core
LCH = GL // 128           # 4 local chunks
NCH = G // 128            # 32 global j-chunks
SLAB = 4                  # j-chunks per sigma slab
W1 = H + 1                # block-1 group width: [1 | h]
W2G = H + 2               # block-2 group width: [s | 1 | h]
ALPHA, BETA, BN_EPS = 0.005, 5e-5, 1e-5

F32 = mybir.dt.float32
BF16 = mybir.dt.bfloat16
AF = mybir.ActivationFunctionType
ALU = mybir.AluOpType
AX = mybir.AxisListType.X

_CACHE = {}




def build_program():
    nc = bacc.Bacc("TRN2", target_bir_lowering=False, debug=False,
                   enable_asserts=False, num_devices=NC)

    def din(name, shape, dt):
        return nc.dram_tensor(name, shape, dt, kind="ExternalInput").ap()

    xT_aug = din("xT_aug", [NI + 1, B * G], F32)           # row 8 = ones
    xT_loc = din("xT_loc", [NI + 1, B * GL], F32)          # row 8 = ones
    w1T = din("w1T", [G, GL], BF16)
    w2T = din("w2T", [G, GL], BF16)
    W_aug = din("W_aug", [NI + 1, H], F32)
    wrep1 = din("wrep1", [128, NCH * H], BF16)
    We1_dst = din("We1_dst", [H + 1, 1], F32)
    We2_dst = din("We2_dst", [H + 1, 1], F32)
    M2 = din("M2", [H + 1, W2G], F32)
    Wn1a = din("Wn1a", [H + 1, NO], F32)
    Wn1b = din("Wn1b", [H + 1, NO], F32)
    Wm1a = din("Wm1a", [H + 1, NO], F32)
    Wm1b = din("Wm1b", [H + 1, NO], F32)
    Wn2a = din("Wn2a", [H + 1, NO], F32)
    Wn2b = din("Wn2b", [H + 1, NO], F32)
    Wm2a = din("Wm2a", [H + 1, NO], F32)
    Wm2b = din("Wm2b", [H + 1, NO], F32)
    bn_g_row = din("bn_g_row", [1, GL], F32)
    bn_b_row = din("bn_b_row", [1, GL], F32)

    out = nc.dram_tensor("out", [NO, B * GL], F32, kind="ExternalOutput").ap()

    with tile.TileContext(nc) as tc:
        with (
            tc.tile_pool(name="cp", bufs=1) as cp,
            tc.tile_pool(name="bp", bufs=1) as bp,
            tc.tile_pool(name="wp", bufs=1) as wp,
            tc.tile_pool(name="sp", bufs=3) as sp,
            tc.tile_pool(name="pp", bufs=1, space="PSUM") as pp,
            tc.tile_pool(name="dp", bufs=1, space="DRAM") as dp,
        ):
            # ---------- constants (small DMAs first) ----------
            sm = {}
            for nm, ap_ in [("W_aug", W_aug), ("We1_dst", We1_dst),
                            ("We2_dst", We2_dst), ("M2", M2),
                            ("Wn1a", Wn1a), ("Wn1b", Wn1b),
                            ("Wm1a", Wm1a), ("Wm1b", Wm1b),
                            ("Wn2a", Wn2a), ("Wn2b", Wn2b),
                            ("Wm2a", Wm2a), ("Wm2b", Wm2b),
                            ("bn_g_row", bn_g_row), ("bn_b_row", bn_b_row),
                            ("wrep1", wrep1)]:
                t = cp.tile(list(ap_.shape), ap_.dtype, name=f"{nm}_sb",
                            tag=f"{nm}_sb")
                nc.sync.dma_start(t[:], ap_[:])
                sm[nm] = t
            ones_r32 = cp.tile([1, H], F32, name="ones_r32", tag="ones_r32")
            nc.vector.memset(ones_r32[:], 1.0)
            ones_r128 = cp.tile([1, 128], F32, name="ones_r128", tag="ones_r128")
            nc.vector.memset(ones_r128[:], 1.0)
            onesk = cp.tile([H, 1], F32, name="onesk", tag="onesk")
            nc.vector.memset(onesk[:], 1.0)
            xTl_sb = cp.tile([NI + 1, B * GL], F32, name="xTl_sb", tag="xTl_sb")
            nc.sync.dma_start(xTl_sb[:], xT_loc[:])
            xTa_sb = cp.tile([NI + 1, B * G], F32, name="xTa_sb", tag="xTa_sb")
            nc.sync.dma_start(xTa_sb[:], xT_aug[:])

            # ---------- big resident tensors ----------
            w1T_sb = bp.tile([128, NCH * GL], BF16, name="w1T_sb", tag="w1T_sb")
            w2T_sb = bp.tile([128, NCH * GL], BF16, name="w2T_sb", tag="w2T_sb")
            w1T_r = w1T.rearrange("(k p) i -> p k i", p=128)
            w2T_r = w2T.rearrange("(k p) i -> p k i", p=128)
            for kq in range(4):
                nc.sync.dma_start(
                    w1T_sb[:, kq * 8 * GL:(kq + 1) * 8 * GL],
                    w1T_r[:, kq * 8:(kq + 1) * 8])

            h0n = bp.tile([128, B * NCH * W1], BF16, name="h0n", tag="h0n")
            ssrc1 = bp.tile([128, B * NCH], F32, name="ssrc1", tag="ssrc1")
            h0l = bp.tile([H + 1, B * GL], F32, name="h0l", tag="h0l")
            h1T = bp.tile([H, B * GL], F32, name="h1T", tag="h1T")
            sqT = bp.tile([H, B * GL], F32, name="sqT", tag="sqT")
            hbnT = bp.tile([H + 1, B * GL], F32, name="hbnT", tag="hbnT")
            h2nat = bp.tile([128, B * LCH * W2G], BF16, name="h2nat",
                            tag="h2nat")
            ghat = [bp.tile([128, NC * LCH * W2G], BF16, name=f"ghat{b}",
                            tag=f"ghat{b}") for b in range(B)]
            out2T = bp.tile([NO, B * GL], F32, name="out2T", tag="out2T")
            nc.vector.memset(h0n[:], 1.0)
            nc.vector.memset(h0l[H:H + 1, :], 1.0)
            nc.vector.memset(hbnT[H:H + 1, :], 1.0)

            gather_in = dp.tile([128, B * LCH * W2G], BF16, name="gin",
                                tag="gin")
            gather_out = dp.tile([NC * 128, B * LCH * W2G], BF16,
                                 addr_space="Shared", name="gout", tag="gout")

            def elu(z_psum, out_ap, shape):
                p, f = shape
                tf = wp.tile([128, GL], F32, name="elu_t", tag="elu_t", bufs=3)
                t1 = tf[0:p, 0:f]
                nc.vector.tensor_scalar_min(t1, z_psum, 0.0)
                nc.scalar.activation(t1, t1, AF.Exp)
                nc.vector.tensor_scalar_add(t1, t1, -1.0)
                nc.vector.tensor_tensor(out_ap, z_psum, t1, op=ALU.max)

            # ---------- phase 1: h0 (fp32) ----------
            h0n_v = h0n.rearrange("p (q e) -> p q e", e=W1)
            for kq in range(8):
                ps = pp.tile([128, 8 * H], F32, name="ps_h0", tag="sm", bufs=2)
                for s in range(8):
                    nc.tensor.matmul(ps[:, s * H:(s + 1) * H],
                                     xTa_sb[:, kq * 1024 + s * 128:
                                            kq * 1024 + (s + 1) * 128],
                                     sm["W_aug"][:], start=True, stop=True)
                elu(ps[:], h0n_v[:, kq * 8:(kq + 1) * 8, 1:W1], [128, 8 * H])
            for b in range(B):
                ps = pp.tile([H, GL], F32, name="ps_h0l", tag="sm", bufs=2)
                nc.tensor.matmul(ps[:], sm["W_aug"][:],
                                 xTl_sb[:, b * GL:(b + 1) * GL],
                                 start=True, stop=True)
                elu(ps[:], h0l[0:H, b * GL:(b + 1) * GL], [H, GL])

            # ---------- ssrc1 (DVE) ----------
            wrep1_v = sm["wrep1"].rearrange("p (q f) -> p q f", f=H)
            for b in range(B):
                ssx = wp.tile([128, NCH * H], BF16, name="ssx", tag="ssx",
                              bufs=2)
                ssx_v = ssx.rearrange("p (q f) -> p q f", f=H)
                nc.vector.tensor_tensor(
                    ssx_v, h0n_v[:, b * NCH:(b + 1) * NCH, 1:W1], wrep1_v,
                    op=ALU.mult)
                nc.vector.reduce_sum(ssrc1[:, b * NCH:(b + 1) * NCH],
                                     ssx_v, axis=AX)

            # ---------- one message-passing block ----------
            def sig_block(blk, wT_sb, sdb, bias_of, stat_of, acc, order):
                # order: list of slab indices; chunks k = q*SLAB..q*SLAB+3
                nq = len(order)
                for b in range(B):
                    first = True
                    for qi, q in enumerate(order):
                        sig = sp.tile([128, SLAB * GL], BF16, name="sig",
                                      tag="sig", bufs=3)
                        for j in range(SLAB):
                            k = q * SLAB + j
                            nc.scalar.activation(
                                sig[:, j * GL:(j + 1) * GL], sdb[b][:],
                                AF.Sigmoid, bias=bias_of(b, k))
                        nc.vector.tensor_tensor(
                            sig[:], sig[:],
                            wT_sb[:, q * SLAB * GL:(q + 1) * SLAB * GL],
                            op=ALU.mult)
                        for j in range(SLAB):
                            k = q * SLAB + j
                            last = (qi == nq - 1) and (j == SLAB - 1)
                            nc.tensor.matmul(
                                acc[b][:], stat_of(b, k),
                                sig[:, j * GL:(j + 1) * GL],
                                start=first, stop=last)
                            first = False

            def sdst_sdb(hT, We_d, tag):
                sd_row = wp.tile([1, B * GL], F32, name=f"sd{tag}",
                                 tag=f"sd{tag}")
                sdbs = []
                for b in range(B):
                    ps_d = pp.tile([1, GL], F32, name="ps_d", tag="sm", bufs=2)
                    nc.tensor.matmul(ps_d[:], We_d[:],
                                     hT[:, b * GL:(b + 1) * GL],
                                     start=True, stop=True)
                    nc.vector.tensor_copy(sd_row[:, b * GL:(b + 1) * GL],
                                          ps_d[:])
                    sdb = pp.tile([128, GL], F32, name=f"sdb{tag}",
                                  tag="sdb", bufs=2)
                    nc.tensor.matmul(sdb[:], ones_r128[:],
                                     sd_row[:, b * GL:(b + 1) * GL],
                                     start=True, stop=True)
                    sdbs.append(sdb)
                return sdbs

            def tail(blk, acc, hT, Wna, Wnb, Wma, Wmb, merge_dst):
                for b in range(B):
                    rfull = wp.tile([H + 1, GL], F32, name="rfull",
                                    tag="rfull", bufs=2)
                    nc.vector.tensor_copy(rfull[:], acc[b][:])
                    ps_rb = pp.tile([H, GL], F32, name="ps_rb", tag="sm",
                                    bufs=2)
                    nc.tensor.matmul(ps_rb[:], ones_r32[:], rfull[0:1, :],
                                     start=True, stop=True)
                    hdT = wp.tile([H + 1, GL], F32, name="hdT", tag="hdT",
                                  bufs=2)
                    nc.vector.tensor_tensor(hdT[0:H, :],
                                            hT[0:H, b * GL:(b + 1) * GL],
                                            ps_rb[:], op=ALU.mult)
                    nc.vector.memset(hdT[H:H + 1, :], 1.0)
                    ps_n = pp.tile([H, GL], F32, name="ps_n", tag="sm", bufs=2)
                    nc.tensor.matmul(ps_n[:], Wna[:], rfull[:],
                                     start=True, stop=False)
                    nc.tensor.matmul(ps_n[:], Wnb[:], hdT[:],
                                     start=False, stop=True)
                    nodesT = wp.tile([H + 1, GL], F32, name="nodesT",
                                     tag="nodesT", bufs=2)
                    elu(ps_n[:], nodesT[0:H, :], [H, GL])
                    nc.vector.memset(nodesT[H:H + 1, :], 1.0)
                    ps_m = pp.tile([H, GL], F32, name="ps_m", tag="sm", bufs=2)
                    nc.tensor.matmul(ps_m[:], Wma[:], nodesT[:],
                                     start=True, stop=False)
                    nc.tensor.matmul(ps_m[:], Wmb[:],
                                     hT[:, b * GL:(b + 1) * GL],
                                     start=False, stop=True)
                    merge_dst(b, ps_m)

            # ---------- block 1 ----------
            sdb1 = sdst_sdb(h0l, sm["We1_dst"], "1")
            acc1 = [pp.tile([H + 1, GL], F32, name=f"acc1{b}", tag=f"acc{b}",
                            bufs=1) for b in range(B)]
            sig_block(1, w1T_sb, sdb1,
                      lambda b, k: ssrc1[:, b * NCH + k:b * NCH + k + 1],
                      lambda b, k: h0n_v[:, b * NCH + k, :],
                      acc1, list(range(8)))
            for kq in range(4):
                nc.sync.dma_start(
                    w2T_sb[:, kq * 8 * GL:(kq + 1) * 8 * GL],
                    w2T_r[:, kq * 8:(kq + 1) * 8])

            def merge1_dst(b, ps_m):
                elu(ps_m[:], h1T[:, b * GL:(b + 1) * GL], [H, GL])

            tail(1, acc1, h0l, sm["Wn1a"], sm["Wn1b"], sm["Wm1a"], sm["Wm1b"],
                 merge1_dst)

            # ---------- BatchNorm (row layout, fully local) ----------
            nc.scalar.activation(sqT[:], h1T[:], AF.Square)
            ps_r0 = pp.tile([1, GL], F32, name="ps_r0", tag="sm", bufs=2)
            for b in range(B):
                nc.tensor.matmul(ps_r0[:], onesk[:],
                                 h1T[:, b * GL:(b + 1) * GL],
                                 start=(b == 0), stop=(b == B - 1))
            ps_r1 = pp.tile([1, GL], F32, name="ps_r1", tag="sm", bufs=2)
            for b in range(B):
                nc.tensor.matmul(ps_r1[:], onesk[:],
                                 sqT[:, b * GL:(b + 1) * GL],
                                 start=(b == 0), stop=(b == B - 1))
            rowb = wp.tile([1, 4 * GL], F32, name="rowb", tag="rowb")
            mu_r, var_r = rowb[:, 0:GL], rowb[:, GL:2 * GL]
            scl_r, shf_r = rowb[:, 2 * GL:3 * GL], rowb[:, 3 * GL:4 * GL]
            nc.vector.tensor_scalar_mul(mu_r, ps_r0[:], 1.0 / (B * NO))
            nc.vector.tensor_scalar_mul(var_r, ps_r1[:], 1.0 / (B * NO))
            nc.vector.tensor_tensor(scl_r, mu_r, mu_r, op=ALU.mult)
            nc.vector.tensor_tensor(var_r, var_r, scl_r, op=ALU.subtract)
            nc.vector.tensor_scalar_add(scl_r, var_r, BN_EPS)
            nc.scalar.activation(scl_r, scl_r, AF.Ln)
            nc.scalar.activation(scl_r, scl_r, AF.Exp, scale=-0.5)
            nc.vector.tensor_tensor(scl_r, scl_r, sm["bn_g_row"][:],
                                    op=ALU.mult)
            nc.vector.tensor_tensor(var_r, mu_r, scl_r, op=ALU.mult)
            nc.vector.tensor_tensor(shf_r, sm["bn_b_row"][:], var_r,
                                    op=ALU.subtract)
            ps_sc = pp.tile([H, GL], F32, name="ps_sc", tag="sm", bufs=2)
            nc.tensor.matmul(ps_sc[:], ones_r32[:], scl_r, start=True,
                             stop=True)
            sc_sb = wp.tile([H, GL], F32, name="sc_sb", tag="sc_sb")
            nc.vector.tensor_copy(sc_sb[:], ps_sc[:])
            ps_sh = pp.tile([H, GL], F32, name="ps_sh", tag="sm", bufs=2)
            nc.tensor.matmul(ps_sh[:], ones_r32[:], shf_r, start=True,
                             stop=True)
            sh_sb = wp.tile([H, GL], F32, name="sh_sb", tag="sh_sb")
            nc.vector.tensor_copy(sh_sb[:], ps_sh[:])
            # BN apply, fused transpose+ssrc2, gather
            for b in range(B):
                sl = slice(b * GL, (b + 1) * GL)
                nc.vector.tensor_tensor(hbnT[0:H, sl], h1T[:, sl], sc_sb[:],
                                        op=ALU.mult)
                nc.vector.tensor_tensor(hbnT[0:H, sl], hbnT[0:H, sl],
                                        sh_sb[:], op=ALU.add)
                for l in range(LCH):
                    ps_t = pp.tile([128, W2G], F32, name="ps_t", tag="sm",
                                   bufs=2)
                    nc.tensor.matmul(ps_t[:],
                                     hbnT[:, b * GL + l * 128:
                                          b * GL + (l + 1) * 128],
                                     sm["M2"][:], start=True, stop=True)
                    q = b * LCH + l
                    nc.vector.tensor_copy(h2nat[:, q * W2G:(q + 1) * W2G],
                                          ps_t[:])
            nc.sync.dma_start(gather_in[:], h2nat[:])
            nc.gpsimd.collective_compute(
                "AllGather", ALU.bypass, replica_groups=[list(range(NC))],
                ins=[gather_in.opt()], outs=[gather_out.opt()])

            # ---------- block 2 prep (overlaps CC transfer) ----------
            sdb2 = sdst_sdb(hbnT, sm["We2_dst"], "2")
            acc2 = [pp.tile([H + 1, GL], F32, name=f"acc2{b}", tag=f"acc{b}",
                            bufs=1) for b in range(B)]
            ghat_v = [g.rearrange("p (q e) -> p q e", e=W2G) for b, g in
                      enumerate(ghat)]

            def bias2(b, k):
                return ghat_v[b][:, k, 0:1]

            def stat2(b, k):
                return ghat_v[b][:, k, 1:W2G]

            for b in range(B):
                for c in range(NC):
                    nc.sync.dma_start(
                        ghat[b][:, c * LCH * W2G:(c + 1) * LCH * W2G],
                        gather_out[c * 128:(c + 1) * 128,
                                   b * LCH * W2G:(b + 1) * LCH * W2G])
            sig_block(2, w2T_sb, sdb2, bias2, stat2, acc2, list(range(8)))

            def merge2_dst(b, ps_m):
                elu(ps_m[:], out2T[:, b * GL:(b + 1) * GL], [H, GL])

            tail(2, acc2, hbnT, sm["Wn2a"], sm["Wn2b"], sm["Wm2a"],
                 sm["Wm2b"], merge2_dst)

            nc.sync.dma_start(out[:], out2T[:])

    nc.compile()
    return nc


def _prep_inputs(x, edges1, edges2, W_infer, b_infer, W_e1, b_e1, W_e2, b_e2,
                 W_n1, b_n1, W_n2, b_n2, W_m1, b_m1, W_m2, b_m2,
                 bn_gamma, bn_beta):
    f32 = np.float32
    bf16 = ml_dtypes.bfloat16
    xT = np.asarray(x, f32).transpose(2, 0, 1).reshape(NI, B * G)
    xT_aug = np.concatenate([xT, np.ones((1, B * G), f32)], axis=0)
    w1 = (ALPHA + (1.0 - ALPHA) * np.asarray(edges1, f32)).astype(bf16)
    w2 = (BETA + (1.0 - BETA) * np.asarray(edges2, f32)).astype(bf16)

    def wecat(W_e, b_e):
        c0 = np.concatenate([np.asarray(W_e, f32)[:H, 0], [0.0]]).astype(f32)
        c1 = np.concatenate([np.asarray(W_e, f32)[H:, 0],
                             [np.asarray(b_e, f32)[0]]]).astype(f32)
        return np.stack([c0, c1], axis=1)

    We1 = wecat(W_e1, b_e1)
    We2 = wecat(W_e2, b_e2)
    z = np.zeros((1, NO), f32)

    def stk(Wpart, brow):
        return np.concatenate([np.asarray(Wpart, f32), brow], 0)

    M2 = np.zeros((H + 1, W2G), f32)
    M2[:H, 0] = We2[:H, 0]
    M2[H, 1] = 1.0
    M2[np.arange(H), 2 + np.arange(H)] = 1.0

    com = dict(
        xT_aug=xT_aug,
        W_aug=np.concatenate([np.asarray(W_infer, f32),
                              np.asarray(b_infer, f32)[None, :]], 0),
        wrep1=np.ascontiguousarray(
            np.broadcast_to(np.tile(We1[:H, 0], NCH)[None, :],
                            (128, NCH * H))).astype(bf16),
        We1_dst=We1[:, 1:2],
        We2_dst=We2[:, 1:2],
        M2=M2,
        Wn1a=np.concatenate([z, np.asarray(W_n1, f32)[:H]], 0),
        Wn1b=stk(np.asarray(W_n1, f32)[H:], np.asarray(b_n1, f32)[None, :]),
        Wm1a=stk(np.asarray(W_m1, f32)[:H], np.asarray(b_m1, f32)[None, :]),
        Wm1b=stk(np.asarray(W_m1, f32)[H:], z),
        Wn2a=np.concatenate([z, np.asarray(W_n2, f32)[:H]], 0),
        Wn2b=stk(np.asarray(W_n2, f32)[H:], np.asarray(b_n2, f32)[None, :]),
        Wm2a=stk(np.asarray(W_m2, f32)[:H], np.asarray(b_m2, f32)[None, :]),
        Wm2b=stk(np.asarray(W_m2, f32)[H:], z),
    )
    in_maps = []
    for c in range(NC):
        sl = slice(c * GL, (c + 1) * GL)
        xl = np.asarray(x, f32)[:, sl, :].transpose(2, 0, 1).reshape(NI, B * GL)
        m = dict(com)
        m["xT_loc"] = np.concatenate([xl, np.ones((1, B * GL), f32)], 0)
        m["w1T"] = np.ascontiguousarray(w1[sl, :].T)
        m["w2T"] = np.ascontiguousarray(w2[sl, :].T)
        m["bn_g_row"] = np.ascontiguousarray(
            np.asarray(bn_gamma, f32)[sl][None, :])
        m["bn_b_row"] = np.ascontiguousarray(
            np.asarray(bn_beta, f32)[sl][None, :])
        in_maps.append(m)
    return in_maps


def kernel(**inputs):
    if "nc" not in _CACHE:
        _CACHE["nc"] = build_program()
    nc = _CACHE["nc"]
    in_maps = _prep_inputs(**inputs)
    res = run_bass_kernel_spmd(nc, in_maps, list(range(NC)))
    parts = [res.results[c]["out"].reshape(NO, B, GL).transpose(1, 2, 0)
             for c in range(NC)]
    return np.concatenate(parts, axis=1).astype(np.float32)


# revision 28
# speedup vs baseline: 1.1146x; 1.1146x over previous
"""Trainium2 Bass kernel for gnn_message_passing (nn_BFR_28089086116615).

Sharding: receiver axis i (G=4096 -> 8 cores x 512). Host pre-transposes the
edge matrices and folds the {coef, 1} gate weights in bf16: wT[j, i]. On
device, sigma^T is computed natively in [j-partition, i-free] layout (ACT
sigmoid, per-partition bias = s_src[j-chunk], input = broadcast s_dst row),
gated by wT on DVE (bf16 2x), and contracted on PE with stationary weights
[1 | h] so the receiver rowsum lands in psum row 0. s_src comes from a DVE
multiply+reduce over the natural-layout h (no PE involvement). BatchNorm is
per-gene -> fully local; two per-batch AllGathers of normalized h between the
blocks so block-2 can start on batch 0 while batch 1 is still in flight.
"""
import sys
sys.path.insert(0, "/opt/trn_rl_repo")
import numpy as np
import ml_dtypes

import concourse.bass as bass
import concourse.bacc as bacc
import concourse.mybir as mybir
import concourse.tile as tile
from concourse.bass_utils import run_bass_kernel_spmd

NC = 8
B, G, NI, H, NO = 2, 4096, 8, 32, 32
GL = G // NC              # 512 local receivers per core
LCH = GL // 128           # 4 local chunks
NCH = G // 128            # 32 global j-chunks
QC = 8                    # j-chunks per sigma quarter-slab
W1 = H + 1                # group width: [1 | h]
ALPHA, BETA, BN_EPS = 0.005, 5e-5, 1e-5

F32 = mybir.dt.float32
BF16 = mybir.dt.bfloat16
AF = mybir.ActivationFunctionType
ALU = mybir.AluOpType
XY = mybir.AxisListType.XY
AX = mybir.AxisListType.X

_CACHE = {}


def build_program():

    nc = bacc.Bacc("TRN2", target_bir_lowering=False, debug=False,
                   enable_asserts=False, num_devices=NC)

    def din(name, shape, dt):
        return nc.dram_tensor(name, shape, dt, kind="ExternalInput").ap()

    xT_aug = din("xT_aug", [NI + 1, B * G], F32)           # row 8 = ones
    xT_loc = din("xT_loc", [NI + 1, B * GL], F32)          # row 8 = ones
    w1T = din("w1T", [G, GL], BF16)
    w2T = din("w2T", [G, GL], BF16)
    W_aug = din("W_aug", [NI + 1, H], F32)
    We1_f = din("We1_f", [H + 1, 2], F32)
    We2_f = din("We2_f", [H + 1, 2], F32)
    We1_rep = din("We1_rep", [1, NCH * H], BF16)
    We2_rep = din("We2_rep", [1, NCH * H], BF16)
    Wn1a = din("Wn1a", [H + 1, NO], F32)                   # [0; W_n[:H]]
    Wn1b = din("Wn1b", [H + 1, NO], F32)                   # [W_n[H:]; b_n]
    Wm1a = din("Wm1a", [H + 1, NO], F32)
    Wm1b = din("Wm1b", [H + 1, NO], F32)
    Wn2a = din("Wn2a", [H + 1, NO], F32)
    Wn2b = din("Wn2b", [H + 1, NO], F32)
    Wm2a = din("Wm2a", [H + 1, NO], F32)
    Wm2b = din("Wm2b", [H + 1, NO], F32)
    bn_g_nat = din("bn_g_nat", [128, LCH], F32)
    bn_b_nat = din("bn_b_nat", [128, LCH], F32)
    bn_g_row = din("bn_g_row", [1, GL], F32)
    bn_b_row = din("bn_b_row", [1, GL], F32)

    out = nc.dram_tensor("out", [B * GL, NO], F32, kind="ExternalOutput").ap()
    out_r = out.rearrange("(b l p) f -> p b l f", b=B, l=LCH, p=128)

    with tile.TileContext(nc) as tc:
        with (
            tc.tile_pool(name="cp", bufs=1) as cp,
            tc.tile_pool(name="bp", bufs=1) as bp,
            tc.tile_pool(name="wp", bufs=1) as wp,
            tc.tile_pool(name="sp", bufs=2) as sp,
            tc.tile_pool(name="pp", bufs=1, space="PSUM") as pp,
            tc.tile_pool(name="dp", bufs=1, space="DRAM") as dp,
        ):
            # ---------- constants (small DMAs first: they gate compute) ----
            W_aug_sb = cp.tile([NI + 1, H], F32, name="W_aug_sb", tag="W_aug_sb")
            nc.sync.dma_start(W_aug_sb[:], W_aug[:])
            sm = {}
            for nm, ap_ in [("We1_rep", We1_rep), ("We2_rep", We2_rep),
                            ("We1_f", We1_f), ("We2_f", We2_f),
                            ("Wn1a", Wn1a), ("Wn1b", Wn1b),
                            ("Wm1a", Wm1a), ("Wm1b", Wm1b),
                            ("Wn2a", Wn2a), ("Wn2b", Wn2b),
                            ("Wm2a", Wm2a), ("Wm2b", Wm2b),
                            ("bn_g_nat", bn_g_nat), ("bn_b_nat", bn_b_nat),
                            ("bn_g_row", bn_g_row), ("bn_b_row", bn_b_row)]:
                t = cp.tile(list(ap_.shape), ap_.dtype, name=f"{nm}_sb",
                            tag=f"{nm}_sb")
                nc.sync.dma_start(t[:], ap_[:])
                sm[nm] = t
            ones_c = cp.tile([1, 128], F32, name="ones_c", tag="ones_c")
            nc.vector.memset(ones_c[:], 1.0)
            ones_cb = cp.tile([1, 128], BF16, name="ones_cb", tag="ones_cb")
            nc.vector.memset(ones_cb[:], 1.0)
            onesk = cp.tile([H, 1], F32, name="onesk", tag="onesk")
            nc.vector.memset(onesk[:], 1.0)
            xTl_sb = cp.tile([NI + 1, B * GL], F32, name="xTl_sb", tag="xTl_sb")
            nc.sync.dma_start(xTl_sb[:], xT_loc[:])

            # ---------- big resident tensors ----------
            h0n = bp.tile([128, B * NCH * W1], BF16, name="h0n", tag="h0n")
            h0l = bp.tile([H + 1, B * GL], F32, name="h0l", tag="h0l")
            nodes1T = bp.tile([H + 1, B * GL], F32, name="nodes1T", tag="nodes1T")
            nodes2T = bp.tile([H + 1, B * GL], F32, name="nodes2T", tag="nodes2T")
            hbnT_f = bp.tile([H + 1, B * GL], F32, name="hbnT_f", tag="hbnT_f")
            ghat = [bp.tile([128, NC * LCH * W1], BF16, name=f"ghat{b}",
                            tag=f"ghat{b}") for b in range(B)]
            nc.vector.memset(h0n[:], 1.0)
            nc.vector.memset(h0l[H:H + 1, :], 1.0)
            nc.vector.memset(nodes1T[H:H + 1, :], 1.0)
            nc.vector.memset(nodes2T[H:H + 1, :], 1.0)
            nc.vector.memset(hbnT_f[H:H + 1, :], 1.0)

            def elu(z_psum, out_ap, shape):
                p, f = shape
                tf = wp.tile([128, GL], F32, name="elu_t", tag="elu_t", bufs=3)
                t1 = tf[0:p, 0:f]
                nc.vector.tensor_scalar_min(t1, z_psum, 0.0)
                nc.scalar.activation(t1, t1, AF.Exp)
                nc.vector.tensor_scalar_add(t1, t1, -1.0)
                nc.vector.tensor_tensor(out_ap, z_psum, t1, op=ALU.max)

            # ---------- phase 1: h0 (natural layout, groups [1|h]) ----------
            h0n_v = h0n.rearrange("p (q e) -> p q e", e=W1)
            for kq in range(8):
                xq = wp.tile([NI + 1, 8 * 128], F32, name="xq", tag="xq", bufs=2)
                nc.sync.dma_start(xq[:], xT_aug[:, kq * 1024:(kq + 1) * 1024])
                ps = pp.tile([128, 8 * H], F32, name="ps_sm", tag="sm", bufs=4)
                for s in range(8):
                    nc.tensor.matmul(ps[:, s * H:(s + 1) * H],
                                     xq[:, s * 128:(s + 1) * 128],
                                     W_aug_sb[:], start=True, stop=True)
                elu(ps[:], h0n_v[:, kq * 8:(kq + 1) * 8, 1:W1], [128, 8 * H])
            for b in range(B):
                ps = pp.tile([H, GL], F32, name="ps_sm", tag="sm", bufs=4)
                nc.tensor.matmul(ps[:], W_aug_sb[:],
                                 xTl_sb[:, b * GL:(b + 1) * GL],
                                 start=True, stop=True)
                elu(ps[:], h0l[0:H, b * GL:(b + 1) * GL], [H, GL])

            # big edge-weight DMAs issued after the gating small ones
            w1T_sb = bp.tile([128, NCH * GL], BF16, name="w1T_sb", tag="w1T_sb")
            w2T_sb = bp.tile([128, NCH * GL], BF16, name="w2T_sb", tag="w2T_sb")
            w1T_r = w1T.rearrange("(k p) i -> p k i", p=128)
            w2T_r = w2T.rearrange("(k p) i -> p k i", p=128)
            for kq in range(4):
                nc.sync.dma_start(
                    w1T_sb[:, kq * QC * GL:(kq + 1) * QC * GL],
                    w1T_r[:, kq * QC:(kq + 1) * QC])

            gather_in = dp.tile([128, B * LCH * W1], BF16, name="gin",
                                tag="gin")
            gather_out = dp.tile([NC * 128, B * LCH * W1], BF16,
                                 addr_space="Shared", name="gout", tag="gout")

            # ---------- one message-passing block ----------
            def mp_block(blk, wT_sb, We_rep, We_f, Wna, Wnb, Wma, Wmb,
                         nat_of, hTl, nodesT, merge_dst):
                # s_src[p, col] = sum_f h_nat[p, g*33+1+f] * We_src[f]  (DVE)
                wrep = wp.tile([128, NCH * H], BF16, name="wrep", tag="wrep",
                               bufs=1)
                for c4 in range(NCH * H // 512):
                    ps_w = pp.tile([128, 512], F32, name="ps_w", tag="bc",
                                   bufs=2)
                    nc.tensor.matmul(ps_w[:], ones_cb[:],
                                     We_rep[:, c4 * 512:(c4 + 1) * 512],
                                     start=True, stop=True)
                    nc.vector.tensor_copy(wrep[:, c4 * 512:(c4 + 1) * 512],
                                          ps_w[:])
                wrep_v = wrep.rearrange("p (q f) -> p q f", f=H)
                ssrc = wp.tile([128, B * NCH], F32, name=f"ssrc{blk}",
                               tag=f"ssrc{blk}")
                for b in range(B):
                    h_nat, goff = nat_of(b)
                    h_nat_v = h_nat.rearrange("p (q e) -> p q e", e=W1)
                    ssx = wp.tile([128, NCH * H], BF16, name="ssx", tag="ssx",
                                  bufs=2)
                    ssx_v = ssx.rearrange("p (q f) -> p q f", f=H)
                    nc.vector.tensor_tensor(
                        ssx_v, h_nat_v[:, goff:goff + NCH, 1:W1], wrep_v,
                        op=ALU.mult)
                    nc.vector.reduce_sum(ssrc[:, b * NCH:(b + 1) * NCH],
                                         ssx_v, axis=AX)
                accs = []
                for b in range(B):
                    h_nat, goff = nat_of(b)
                    h_nat_v = h_nat.rearrange("p (q e) -> p q e", e=W1)
                    ps_d = pp.tile([1, GL], F32, name="ps_d", tag="sm", bufs=4)
                    nc.tensor.matmul(ps_d[:], We_f[:, 1:2],
                                     hTl[:, b * GL:(b + 1) * GL],
                                     start=True, stop=True)
                    sd_row = wp.tile([1, GL], F32, name="sd_row", tag="sd_row",
                                     bufs=2)
                    nc.vector.tensor_copy(sd_row[:], ps_d[:])
                    ps_bc = pp.tile([128, GL], F32, name="ps_bc", tag="bc",
                                    bufs=2)
                    nc.tensor.matmul(ps_bc[:], ones_c[:], sd_row[:],
                                     start=True, stop=True)
                    sdb = wp.tile([128, GL], F32, name="sdb", tag="sdb", bufs=2)
                    nc.vector.tensor_copy(sdb[:], ps_bc[:])

                    ps_acc = pp.tile([W1, GL], F32, name="ps_acc", tag="acc",
                                     bufs=2)
                    for qq in range(NCH // QC):
                        sig = sp.tile([128, QC * GL], BF16, name="sig",
                                      tag="sig", bufs=2)
                        for k8 in range(QC):
                            k = qq * QC + k8
                            nc.scalar.activation(
                                sig[:, k8 * GL:(k8 + 1) * GL], sdb[:],
                                AF.Sigmoid,
                                bias=ssrc[:, b * NCH + k:b * NCH + k + 1])
                        for hh in range(QC // 4):
                            sl = slice(hh * 4 * GL, (hh + 1) * 4 * GL)
                            wsl = slice((qq * QC + hh * 4) * GL,
                                        (qq * QC + hh * 4 + 4) * GL)
                            nc.vector.tensor_tensor(sig[:, sl], sig[:, sl],
                                                    wT_sb[:, wsl], op=ALU.mult)
                        for k8 in range(QC):
                            k = qq * QC + k8
                            nc.tensor.matmul(
                                ps_acc[:], h_nat_v[:, goff + k, :],
                                sig[:, k8 * GL:(k8 + 1) * GL],
                                start=(k == 0), stop=(k == NCH - 1))
                    accs.append(ps_acc)
                for b in range(B):
                    ps_acc = accs[b]
                    # rows: 0 = rowsum, 1..32 = recv_srcT
                    rfull = wp.tile([H + 1, GL], F32, name="rfull", tag="rfull",
                                    bufs=2)
                    nc.vector.tensor_copy(rfull[:], ps_acc[:])
                    ps_rb = pp.tile([H, GL], F32, name="ps_rb", tag="bc", bufs=2)
                    nc.tensor.matmul(ps_rb[:], ones_c[:, 0:H], rfull[0:1, :],
                                     start=True, stop=True)
                    hdT = wp.tile([H + 1, GL], F32, name="hdT", tag="hdT",
                                  bufs=2)
                    nc.vector.tensor_tensor(hdT[0:H, :],
                                            hTl[0:H, b * GL:(b + 1) * GL],
                                            ps_rb[:], op=ALU.mult)
                    nc.vector.memset(hdT[H:H + 1, :], 1.0)
                    ps_n = pp.tile([H, GL], F32, name="ps_n", tag="sm", bufs=4)
                    nc.tensor.matmul(ps_n[:], Wna[:], rfull[:],
                                     start=True, stop=False)
                    nc.tensor.matmul(ps_n[:], Wnb[:], hdT[:],
                                     start=False, stop=True)
                    elu(ps_n[:], nodesT[0:H, b * GL:(b + 1) * GL], [H, GL])
                    ps_m = pp.tile([128, LCH * NO], F32, name="ps_m", tag="sm",
                                   bufs=4)
                    for l in range(LCH):
                        c0 = b * GL + l * 128
                        nc.tensor.matmul(ps_m[:, l * NO:(l + 1) * NO],
                                         nodesT[:, c0:c0 + 128],
                                         Wma[:], start=True, stop=False)
                        nc.tensor.matmul(ps_m[:, l * NO:(l + 1) * NO],
                                         hTl[:, c0:c0 + 128],
                                         Wmb[:], start=False, stop=True)
                    merge_dst(b, ps_m)

            # ---------- block 1 ----------
            h1n = wp.tile([128, B * LCH * NO], F32, name="h1n", tag="h1n")

            def merge1_dst(b, ps_m):
                c0 = b * LCH * NO
                elu(ps_m[:], h1n[:, c0:c0 + LCH * NO], [128, LCH * NO])

            mp_block(1, w1T_sb, sm["We1_rep"], sm["We1_f"],
                     sm["Wn1a"], sm["Wn1b"], sm["Wm1a"], sm["Wm1b"],
                     lambda b: (h0n, b * NCH), h0l, nodes1T, merge1_dst)
            for kq in range(4):
                nc.sync.dma_start(
                    w2T_sb[:, kq * QC * GL:(kq + 1) * QC * GL],
                    w2T_r[:, kq * QC:(kq + 1) * QC])

            h1T = wp.tile([H, B * GL], F32, name="h1T", tag="h1T")
            for b in range(B):
                ps = pp.tile([H, GL], F32, name="ps_sm2", tag="sm", bufs=4)
                nc.tensor.matmul(ps[:], sm["Wm1a"][:],
                                 nodes1T[:, b * GL:(b + 1) * GL],
                                 start=True, stop=False)
                nc.tensor.matmul(ps[:], sm["Wm1b"][:],
                                 h0l[:, b * GL:(b + 1) * GL],
                                 start=False, stop=True)
                elu(ps[:], h1T[:, b * GL:(b + 1) * GL], [H, GL])

            # ---------- BatchNorm (fully local) ----------
            stat = wp.tile([128, 6 * LCH], F32, name="stat", tag="stat")
            mu_n, var_n = stat[:, 0:LCH], stat[:, LCH:2 * LCH]
            scl_n, shf_n = stat[:, 2 * LCH:3 * LCH], stat[:, 3 * LCH:4 * LCH]
            t_n, t2_n = stat[:, 4 * LCH:5 * LCH], stat[:, 5 * LCH:6 * LCH]
            sq_n = wp.tile([128, B * LCH * NO], F32, name="sq_n", tag="sq_n")
            nc.scalar.activation(sq_n[:], h1n[:], AF.Square)
            h1n_r = h1n.rearrange("p (b l f) -> p b l f", b=B, l=LCH)
            sq_r = sq_n.rearrange("p (b l f) -> p b l f", b=B, l=LCH)
            for l in range(LCH):
                nc.vector.reduce_sum(mu_n[:, l:l + 1], h1n_r[:, :, l, :], axis=XY)
                nc.vector.reduce_sum(var_n[:, l:l + 1], sq_r[:, :, l, :], axis=XY)
            nc.vector.tensor_scalar_mul(mu_n, mu_n, 1.0 / (B * NO))
            nc.vector.tensor_scalar_mul(var_n, var_n, 1.0 / (B * NO))
            nc.vector.tensor_tensor(t_n, mu_n, mu_n, op=ALU.mult)
            nc.vector.tensor_tensor(var_n, var_n, t_n, op=ALU.subtract)
            nc.vector.tensor_scalar_add(t_n, var_n, BN_EPS)
            nc.scalar.activation(t_n, t_n, AF.Ln)
            nc.scalar.activation(t_n, t_n, AF.Exp, scale=-0.5)
            nc.vector.tensor_tensor(scl_n, t_n, sm["bn_g_nat"][:], op=ALU.mult)
            nc.vector.tensor_tensor(t2_n, mu_n, scl_n, op=ALU.mult)
            nc.vector.tensor_tensor(shf_n, sm["bn_b_nat"][:], t2_n,
                                    op=ALU.subtract)
            # normalized h, natural groups [1|h]; per-b gather as soon as ready
            hbn_n = wp.tile([128, B * LCH * W1], BF16, name="hbn_n",
                            tag="hbn_n")
            nc.vector.memset(hbn_n[:], 1.0)
            for b in range(B):
                for l in range(LCH):
                    q = b * LCH + l
                    nc.vector.tensor_scalar(
                        hbn_n[:, q * W1 + 1:(q + 1) * W1],
                        h1n[:, (b * LCH + l) * NO:(b * LCH + l + 1) * NO],
                        scl_n[:, l:l + 1], shf_n[:, l:l + 1],
                        op0=ALU.mult, op1=ALU.add)
            nc.sync.dma_start(gather_in[:], hbn_n[:])
            nc.gpsimd.collective_compute(
                "AllGather", ALU.bypass, replica_groups=[list(range(NC))],
                ins=[gather_in.opt()], outs=[gather_out.opt()])
            for b in range(B):
                for c in range(NC):
                    nc.sync.dma_start(
                        ghat[b][:, c * LCH * W1:(c + 1) * LCH * W1],
                        gather_out[c * 128:(c + 1) * 128,
                                   b * LCH * W1:(b + 1) * LCH * W1])

            # row-layout stats for the feature-major copy
            rowb = wp.tile([1, 4 * GL], F32, name="rowb", tag="rowb")
            mu_r, var_r = rowb[:, 0:GL], rowb[:, GL:2 * GL]
            scl_r, shf_r = rowb[:, 2 * GL:3 * GL], rowb[:, 3 * GL:4 * GL]
            t_r, t2_r = scl_r, shf_r
            sqT = wp.tile([H, B * GL], F32, name="sqT", tag="sqT")
            nc.scalar.activation(sqT[:], h1T[:], AF.Square)
            ps_r0 = pp.tile([1, GL], F32, name="ps_r0", tag="sm", bufs=4)
            for b in range(B):
                nc.tensor.matmul(ps_r0[:], onesk[:],
                                 h1T[:, b * GL:(b + 1) * GL],
                                 start=(b == 0), stop=(b == B - 1))
            ps_r1 = pp.tile([1, GL], F32, name="ps_r1", tag="sm", bufs=4)
            for b in range(B):
                nc.tensor.matmul(ps_r1[:], onesk[:],
                                 sqT[:, b * GL:(b + 1) * GL],
                                 start=(b == 0), stop=(b == B - 1))
            nc.vector.tensor_scalar_mul(mu_r, ps_r0[:], 1.0 / (B * NO))
            nc.vector.tensor_scalar_mul(var_r, ps_r1[:], 1.0 / (B * NO))
            nc.vector.tensor_tensor(t_r, mu_r, mu_r, op=ALU.mult)
            nc.vector.tensor_tensor(var_r, var_r, t_r, op=ALU.subtract)
            nc.vector.tensor_scalar_add(t_r, var_r, BN_EPS)
            nc.scalar.activation(t_r, t_r, AF.Ln)
            nc.scalar.activation(t_r, t_r, AF.Exp, scale=-0.5)
            nc.vector.tensor_tensor(scl_r, t_r, sm["bn_g_row"][:], op=ALU.mult)
            nc.vector.tensor_tensor(t2_r, mu_r, scl_r, op=ALU.mult)
            nc.vector.tensor_tensor(shf_r, sm["bn_b_row"][:], t2_r,
                                    op=ALU.subtract)
            ps_sc = pp.tile([H, GL], F32, name="ps_sc", tag="bc", bufs=2)
            nc.tensor.matmul(ps_sc[:], ones_c[:, 0:H], scl_r, start=True,
                             stop=True)
            ps_sh = pp.tile([H, GL], F32, name="ps_sh", tag="bc", bufs=2)
            nc.tensor.matmul(ps_sh[:], ones_c[:, 0:H], shf_r, start=True,
                             stop=True)
            for b in range(B):
                sl = slice(b * GL, (b + 1) * GL)
                nc.vector.tensor_tensor(hbnT_f[0:H, sl], h1T[:, sl], ps_sc[:],
                                        op=ALU.mult)
                nc.vector.tensor_tensor(hbnT_f[0:H, sl], hbnT_f[0:H, sl],
                                        ps_sh[:], op=ALU.add)

            # ---------- block 2 ----------
            out_n = wp.tile([128, B * LCH * NO], F32, name="out_n", tag="out_n")

            def merge2_dst(b, ps_m):
                c0 = b * LCH * NO
                elu(ps_m[:], out_n[:, c0:c0 + LCH * NO], [128, LCH * NO])

            mp_block(2, w2T_sb, sm["We2_rep"], sm["We2_f"],
                     sm["Wn2a"], sm["Wn2b"], sm["Wm2a"], sm["Wm2b"],
                     lambda b: (ghat[b], 0), hbnT_f, nodes2T, merge2_dst)

            nc.sync.dma_start(out_r, out_n[:])

    nc.compile()
    return nc


def _prep_inputs(x, edges1, edges2, W_infer, b_infer, W_e1, b_e1, W_e2, b_e2,
                 W_n1, b_n1, W_n2, b_n2, W_m1, b_m1, W_m2, b_m2,
                 bn_gamma, bn_beta):
    f32 = np.float32
    bf16 = ml_dtypes.bfloat16
    xT = np.asarray(x, f32).transpose(2, 0, 1).reshape(NI, B * G)
    xT_aug = np.concatenate([xT, np.ones((1, B * G), f32)], axis=0)
    w1 = (ALPHA + (1.0 - ALPHA) * np.asarray(edges1, f32)).astype(bf16)
    w2 = (BETA + (1.0 - BETA) * np.asarray(edges2, f32)).astype(bf16)

    def wecat(W_e, b_e):
        c0 = np.concatenate([np.asarray(W_e, f32)[:H, 0], [0.0]]).astype(f32)
        c1 = np.concatenate([np.asarray(W_e, f32)[H:, 0],
                             [np.asarray(b_e, f32)[0]]]).astype(f32)
        return np.stack([c0, c1], axis=1)

    We1 = wecat(W_e1, b_e1)
    We2 = wecat(W_e2, b_e2)
    z = np.zeros((1, NO), f32)

    def stk(Wpart, brow):
        return np.concatenate([np.asarray(Wpart, f32), brow], 0)

    com = dict(
        xT_aug=xT_aug,
        W_aug=np.concatenate([np.asarray(W_infer, f32),
                              np.asarray(b_infer, f32)[None, :]], 0),
        We1_rep=np.tile(We1[:H, 0], NCH)[None, :].astype(bf16),
        We2_rep=np.tile(We2[:H, 0], NCH)[None, :].astype(bf16),
        We1_f=We1, We2_f=We2,
        Wn1a=np.concatenate([z, np.asarray(W_n1, f32)[:H]], 0),
        Wn1b=stk(np.asarray(W_n1, f32)[H:], np.asarray(b_n1, f32)[None, :]),
        Wm1a=stk(np.asarray(W_m1, f32)[:H], np.asarray(b_m1, f32)[None, :]),
        Wm1b=stk(np.asarray(W_m1, f32)[H:], z),
        Wn2a=np.concatenate([z, np.asarray(W_n2, f32)[:H]], 0),
        Wn2b=stk(np.asarray(W_n2, f32)[H:], np.asarray(b_n2, f32)[None, :]),
        Wm2a=stk(np.asarray(W_m2, f32)[:H], np.asarray(b_m2, f32)[None, :]),
        Wm2b=stk(np.asarray(W_m2, f32)[H:], z),
    )
    in_maps = []
    for c in range(NC):
        sl = slice(c * GL, (c + 1) * GL)
        xl = np.asarray(x, f32)[:, sl, :].transpose(2, 0, 1).reshape(NI, B * GL)
        m = dict(com)
        m["xT_loc"] = np.concatenate([xl, np.ones((1, B * GL), f32)], 0)
        m["w1T"] = np.ascontiguousarray(w1[sl, :].T)
        m["w2T"] = np.ascontiguousarray(w2[sl, :].T)
        g = np.asarray(bn_gamma, f32)[sl]
        b_ = np.asarray(bn_beta, f32)[sl]
        m["bn_g_nat"] = np.ascontiguousarray(g.reshape(LCH, 128).T)
        m["bn_b_nat"] = np.ascontiguousarray(b_.reshape(LCH, 128).T)
        m["bn_g_row"] = np.ascontiguousarray(g[None, :])
        m["bn_b_row"] = np.ascontiguousarray(b_[None, :])
        in_maps.append(m)
    return in_maps


def kernel(**inputs):
    if "nc" not in _CACHE:
        _CACHE["nc"] = build_program()
    nc = _CACHE["nc"]
    in_maps = _prep_inputs(**inputs)
    res = run_bass_kernel_spmd(nc, in_maps, list(range(NC)))
    parts = [res.results[c]["out"].reshape(B, GL, NO) for c in range(NC)]
    return np.concatenate(parts, axis=1).astype(np.float32)
